# revision 18
# baseline (speedup 1.0000x reference)
"""GATv2 classifier kernel for Trainium2, 8-core SPMD.

Strategy:
  - Nodes are dealt round-robin by descending in-degree across 8 cores
    (balances per-core edge counts and makes per-bucket degrees uniform).
  - Edges are partitioned by destination node -> segment softmax and the
    weighted aggregation stay core-local. No collectives: every core
    redundantly computes the full xl = x@Wl+bl table (cheap matmul).
  - Per core, dst nodes are processed in buckets of 128 (partitions) with
    K padded incoming-edge slots (K = per-bucket max degree, rounded to 4).
  - xl rows for edge sources are fetched with dma_gather (int16 indices).
    Since N=50000 exceeds int16 range, xl is stored as TWO DRAM tables
    split at node LO=63*512, each with a zero row at index 0; every slot
    gathers from both tables (invalid side -> index 0 -> zeros) and the
    two results are summed.
  - z = xl[src] + xr[dst] is built entirely on the TensorEngine in PSUM:
    two identity matmuls accumulate the two gathered tables, a third
    matmul (lhsT = xrT chunk, rhs = replicated identity) adds xr.
  - e = sum_f att * leaky_relu(z); p = exp(e) * mask  (no max-subtract:
    |e| <~ 6 for these inputs, exp is safe in fp32/fp16).
  - h = (sum_k p*z) / (sum_k p) - xr   (algebraically removes the xr
    contribution, so gathered xl never needs to be materialized alone).
  - out = sigmoid(h @ Wo + bo'), bo' = bo + bias@Wo (bias folded on host).
"""

import math
import os
import sys

import numpy as np

if os.path.isdir("/opt/trn_rl_repo") and "/opt/trn_rl_repo" not in sys.path:
    sys.path.insert(0, "/opt/trn_rl_repo")

P = 128
NEG_SLOPE = 0.2
CHUNK = 512  # nodes per phase-1 table-build chunk


# --------------------------------------------------------------------------
# Host-side planning
# --------------------------------------------------------------------------

def _plan(x, edge_index, Wl, bl, Wr, br, att, bias, Wo, bo, n_cores=8):
    N, F = x.shape
    assert F == P
    E = edge_index.shape[1]
    C = n_cores

    src = np.concatenate([np.asarray(edge_index[0], dtype=np.int64),
                          np.arange(N, dtype=np.int64)])
    dst = np.concatenate([np.asarray(edge_index[1], dtype=np.int64),
                          np.arange(N, dtype=np.int64)])
    deg = np.bincount(dst, minlength=N)

    # CSR by destination
    e_order = np.argsort(dst, kind="stable")
    src_sorted = src[e_order]
    starts = np.concatenate([[0], np.cumsum(deg)]).astype(np.int64)

    # deal nodes round-robin by descending degree
    order = np.argsort(-deg, kind="stable")
    npc = (N + C - 1) // C
    NB = (npc + P - 1) // P
    npc_pad = NB * P
    order_pad = np.full(C * npc_pad, -1, dtype=np.int64)
    order_pad[: N] = order
    core_nodes = np.stack([order_pad[c::C] for c in range(C)])  # [C, npc_pad]

    # common K schedule (max over cores per bucket, rounded up to mult of 4)
    Ks = []
    for b in range(NB):
        sl = core_nodes[:, b * P:(b + 1) * P]
        valid = sl >= 0
        kmax = int(deg[np.maximum(sl, 0)][valid].max()) if valid.any() else 1
        Ks.append(max(4, ((kmax + 3) // 4) * 4))

    # table geometry
    n_chunks = (N + CHUNK - 1) // CHUNK
    N_pad = n_chunks * CHUNK
    L_lo = min(n_chunks, 63)          # chunks 0..L_lo-1 go to the lo table
    LO = L_lo * CHUNK                 # first node id in the hi table
    lo_rows = LO + 1                  # + zero row
    hi_rows = max(N_pad - LO, 1) + 1

    soffs, koffs = [], []
    s_acc = k_acc = 0
    for K in Ks:
        soffs.append(s_acc)
        koffs.append(k_acc)
        s_acc += (K * P) // 16
        k_acc += K
    Stot16, Ktot = s_acc, k_acc

    # per-core slot tables
    idx_lo = np.zeros((C, P, Stot16), dtype=np.int16)
    idx_hi = np.zeros((C, P, Stot16), dtype=np.int16)
    masks = np.zeros((C, P, Ktot), dtype=np.float16)
    for c in range(C):
        nodes = core_nodes[c]
        for b in range(NB):
            K = Ks[b]
            nb = nodes[b * P:(b + 1) * P]
            db = np.where(nb >= 0, deg[np.maximum(nb, 0)], 0)
            kk = np.arange(K)[:, None]                      # [K, 128]
            valid = kk < db[None, :]
            pos = starts[np.maximum(nb, 0)][None, :] + kk
            srcg = np.where(valid, src_sorted[np.minimum(pos, len(src_sorted) - 1)], 0)
            lo = np.where(valid & (srcg < LO), srcg + 1, 0)
            hi = np.where(valid & (srcg >= LO), srcg - LO + 1, 0)
            m = valid.astype(np.float16)
            dummy = nb < 0
            m[0, dummy] = 1.0        # keep denominator nonzero for pad nodes
            flat_lo = lo.reshape(-1).astype(np.int16)
            flat_hi = hi.reshape(-1).astype(np.int16)
            wr_lo = np.tile(flat_lo.reshape(-1, 16).T, (8, 1))   # [128, S/16]
            wr_hi = np.tile(flat_hi.reshape(-1, 16).T, (8, 1))
            so, ko = soffs[b], koffs[b]
            idx_lo[c, :, so:so + (K * P) // 16] = wr_lo
            idx_hi[c, :, so:so + (K * P) // 16] = wr_hi
            masks[c, :, ko:ko + K] = m.T

    xT_full = np.zeros((P, N_pad), dtype=np.float16)
    xT_full[:, :N] = np.asarray(x, dtype=np.float16).T
    xT_local = np.zeros((C, P, npc_pad), dtype=np.float16)
    for c in range(C):
        nodes = core_nodes[c]
        ok = nodes >= 0
        xT_local[c][:, ok] = np.asarray(x, dtype=np.float16).T[:, nodes[ok]]

    wl = np.asarray(Wl, dtype=np.float16)
    wr = np.asarray(Wr, dtype=np.float16)
    bl_row = np.asarray(bl, dtype=np.float16).reshape(1, P)
    br_row = np.asarray(br, dtype=np.float16).reshape(1, P)
    att16 = np.asarray(att, dtype=np.float16)
    att_rep = np.tile(att16[None, None, :], (P, 4, 1))           # [128,4,128]
    i_rep = np.tile(np.eye(P, dtype=np.float16)[:, None, :], (1, 4, 1))
    wo_rep = np.tile(np.asarray(Wo, dtype=np.float16)[:, 0][None, :], (P, 1))
    bo_eff = float(np.asarray(bo).reshape(-1)[0] +
                   np.asarray(bias, dtype=np.float64) @ np.asarray(Wo, dtype=np.float64)[:, 0])

    cfg = dict(N=N, C=C, NB=NB, npc_pad=npc_pad, Ks=Ks, soffs=soffs,
               koffs=koffs, Stot16=Stot16, Ktot=Ktot, n_chunks=n_chunks,
               N_pad=N_pad, L_lo=L_lo, LO=LO, lo_rows=lo_rows,
               hi_rows=hi_rows, bo_eff=bo_eff)

    in_maps = []
    for c in range(C):
        in_maps.append({
            "xT_full": xT_full,
            "xT_local": np.ascontiguousarray(xT_local[c]),
            "idx_lo": np.ascontiguousarray(idx_lo[c]),
            "idx_hi": np.ascontiguousarray(idx_hi[c]),
            "mask": np.ascontiguousarray(masks[c]),
            "wl": wl, "wr": wr, "bl_row": bl_row, "br_row": br_row,
            "att_rep": att_rep, "i_rep": i_rep, "wo_rep": wo_rep,
        })
    return cfg, in_maps, core_nodes


# --------------------------------------------------------------------------
# Device program
# --------------------------------------------------------------------------

def _build(cfg, lrelu_act=True, debug=False, no_gather=False, stage=9):
    import concourse.bass as bass
    import concourse.bacc as bacc
    import concourse.tile as tile
    from concourse import mybir

    f16, f32, i16 = mybir.dt.float16, mybir.dt.float32, mybir.dt.int16
    AT = mybir.ActivationFunctionType
    OP = mybir.AluOpType
    AX = mybir.AxisListType

    NB, Ks = cfg["NB"], cfg["Ks"]
    n_chunks, L_lo = cfg["n_chunks"], cfg["L_lo"]
    npc_pad = cfg["npc_pad"]

    nc = bacc.Bacc("TRN2", target_bir_lowering=False, debug=debug,
                   num_devices=cfg["C"], num_swdge_queues=2)

    xT_full = nc.dram_tensor("xT_full", [P, cfg["N_pad"]], f16, kind="ExternalInput")
    xT_local = nc.dram_tensor("xT_local", [P, npc_pad], f16, kind="ExternalInput")
    idx_lo_d = nc.dram_tensor("idx_lo", [P, cfg["Stot16"]], i16, kind="ExternalInput")
    idx_hi_d = nc.dram_tensor("idx_hi", [P, cfg["Stot16"]], i16, kind="ExternalInput")
    mask_d = nc.dram_tensor("mask", [P, cfg["Ktot"]], f16, kind="ExternalInput")
    wl_d = nc.dram_tensor("wl", [P, P], f16, kind="ExternalInput")
    wr_d = nc.dram_tensor("wr", [P, P], f16, kind="ExternalInput")
    blr_d = nc.dram_tensor("bl_row", [1, P], f16, kind="ExternalInput")
    brr_d = nc.dram_tensor("br_row", [1, P], f16, kind="ExternalInput")
    attr_d = nc.dram_tensor("att_rep", [P, 4, P], f16, kind="ExternalInput")
    irep_d = nc.dram_tensor("i_rep", [P, 4, P], f16, kind="ExternalInput")
    wo_d = nc.dram_tensor("wo_rep", [P, P], f16, kind="ExternalInput")
    out_d = nc.dram_tensor("out", [npc_pad, 1], f32, kind="ExternalOutput")

    table_lo = nc.dram_tensor("table_lo", [cfg["lo_rows"], P], f16)
    table_hi = nc.dram_tensor("table_hi", [cfg["hi_rows"], P], f16)

    def bc(ap, pattern):
        """Build an AP with an explicit [stride, size] free pattern."""
        return bass.AP(tensor=ap.tensor, offset=ap.offset,
                       ap=[list(ap.ap[0])] + [list(p) for p in pattern])

    with tile.TileContext(nc) as tc:
        with tc.tile_pool(name="const", bufs=1) as cp:
            wl_sb = cp.tile([P, P], f16, tag="wl")
            wr_sb = cp.tile([P, P], f16, tag="wr")
            blr_sb = cp.tile([1, P], f16, tag="blr")
            brr_sb = cp.tile([1, P], f16, tag="brr")
            att_sb = cp.tile([P, 4, P], f16, tag="attr")
            irep_sb = cp.tile([P, 4, P], f16, tag="irep")
            wo_sb = cp.tile([P, P], f16, tag="wo")
            idxlo_sb = cp.tile([P, cfg["Stot16"]], i16, tag="idxlo")
            idxhi_sb = cp.tile([P, cfg["Stot16"]], i16, tag="idxhi")
            mask_sb = cp.tile([P, cfg["Ktot"]], f16, tag="mask")
            xrT_all = cp.tile([P, NB, P], f16, tag="xrT")
            xr_all = cp.tile([P, NB, P], f16, tag="xr")
            ones1 = cp.tile([1, P], f16, tag="ones1")
            zrow = cp.tile([1, P], f16, tag="zrow")
            bo_sb = cp.tile([P, 1], f32, tag="bo")
            out_sb = cp.tile([P, NB], f32, tag="outsb")

            nc.sync.dma_start(out=wl_sb, in_=wl_d.ap())
            nc.sync.dma_start(out=wr_sb, in_=wr_d.ap())
            nc.sync.dma_start(out=blr_sb, in_=blr_d.ap())
            nc.sync.dma_start(out=brr_sb, in_=brr_d.ap())
            nc.sync.dma_start(out=att_sb, in_=attr_d.ap())
            nc.sync.dma_start(out=irep_sb, in_=irep_d.ap())
            nc.sync.dma_start(out=wo_sb, in_=wo_d.ap())
            nc.sync.dma_start(out=idxlo_sb, in_=idx_lo_d.ap())
            nc.sync.dma_start(out=idxhi_sb, in_=idx_hi_d.ap())
            nc.sync.dma_start(out=mask_sb, in_=mask_d.ap())
            nc.vector.memset(ones1, 1.0)
            nc.vector.memset(zrow, 0.0)
            nc.vector.memset(bo_sb, cfg["bo_eff"])
            nc.sync.dma_start(out=table_lo.ap()[0:1, :], in_=zrow)
            nc.sync.dma_start(out=table_hi.ap()[0:1, :], in_=zrow)
            if n_chunks <= L_lo:
                # no hi chunks: zero the hi table's only data row
                nc.sync.dma_start(out=table_hi.ap()[1:2, :], in_=zrow)

            i128 = irep_sb[:, 0, :]

            # ---------------- phase 1a: local xrT / xr chunks ----------------
            with tc.tile_pool(name="p1l", bufs=3) as lp, \
                 tc.tile_pool(name="p1lp", bufs=4, space="PSUM") as lpp:
                for b in range(NB):
                    xtl = lp.tile([P, P], f16, tag="xtl")
                    nc.sync.dma_start(out=xtl, in_=xT_local.ap()[:, b * P:(b + 1) * P])
                    ps1 = lpp.tile([P, P], f32, tag="ps1")
                    nc.tensor.matmul(ps1, wr_sb, xtl, start=True, stop=False)
                    nc.tensor.matmul(ps1, brr_sb, ones1, start=False, stop=True)
                    nc.scalar.copy(xrT_all[:, b, :], ps1)
                    ps2 = lpp.tile([P, P], f32, tag="ps2")
                    nc.tensor.matmul(ps2, xtl, wr_sb, start=True, stop=False)
                    nc.tensor.matmul(ps2, ones1, brr_sb, start=False, stop=True)
                    nc.vector.tensor_copy(xr_all[:, b, :], ps2)

            # ---------------- phase 1b: xl tables ----------------
            with tc.tile_pool(name="p1x", bufs=3) as xp, \
                 tc.tile_pool(name="p1p", bufs=4, space="PSUM") as pp, \
                 tc.tile_pool(name="p1c", bufs=3) as cvp:
                for ch in range(n_chunks):
                    xt = xp.tile([P, 4, P], f16, tag="xt")
                    nc.sync.dma_start(
                        out=xt, in_=xT_full.ap()[:, ch * CHUNK:(ch + 1) * CHUNK])
                    ps = pp.tile([P, 4, P], f32, tag="pch")
                    for i in range(4):
                        nc.tensor.matmul(ps[:, i, :], xt[:, i, :], wl_sb,
                                         start=True, stop=False)
                        nc.tensor.matmul(ps[:, i, :], ones1, blr_sb,
                                         start=False, stop=True)
                    cv = cvp.tile([P, 4, P], f16, tag="cv")
                    if ch % 2 == 0:
                        nc.scalar.copy(cv, ps)
                    else:
                        nc.vector.tensor_copy(cv, ps)
                    if ch < L_lo:
                        r0 = ch * CHUNK + 1
                        dst = table_lo.ap()[r0:r0 + CHUNK, :]
                    else:
                        r0 = ch * CHUNK - cfg["LO"] + 1
                        dst = table_hi.ap()[r0:r0 + CHUNK, :]
                    nc.sync.dma_start(
                        out=dst.rearrange("(i n) f -> n i f", n=P), in_=cv)

            # ---------------- phase 2: per-bucket GAT ----------------
            if stage < 2:
                nc.vector.memset(out_sb, 0.5)
            with tc.tile_pool(name="gat", bufs=2) as gp, \
                 tc.tile_pool(name="zps", bufs=4, space="PSUM") as zp, \
                 tc.tile_pool(name="sm", bufs=3) as sp:
                for b in range(NB if stage >= 2 else 0):
                    K = Ks[b]
                    S = K * P
                    nb4 = K // 4
                    so, ko = cfg["soffs"][b], cfg["koffs"][b]
                    glo = gp.tile([P, K, P], f16, tag="glo")
                    ghi = gp.tile([P, K, P], f16, tag="ghi")
                    if no_gather:
                        nc.vector.memset(glo, 0.0)
                        nc.vector.memset(ghi, 0.0)
                    else:
                        # Q7 gather scratch caps num_idxs at 1024 (8 slot-cols)
                        for j0 in range(0, K, 8):
                            kc = min(8, K - j0)
                            sc = kc * P
                            nc.gpsimd.dma_gather(
                                out_ap=glo[:, j0:j0 + kc, :],
                                in_ap=table_lo.ap(),
                                idxs_ap=idxlo_sb[:, so + j0 * 8:so + j0 * 8 + sc // 16],
                                num_idxs=sc, num_idxs_reg=sc, elem_size=P,
                                queue_num=0)
                            nc.gpsimd.dma_gather(
                                out_ap=ghi[:, j0:j0 + kc, :],
                                in_ap=table_hi.ap(),
                                idxs_ap=idxhi_sb[:, so + j0 * 8:so + j0 * 8 + sc // 16],
                                num_idxs=sc, num_idxs_reg=sc, elem_size=P,
                                queue_num=1)
                    lr = gp.tile([P, K, P], f16, tag="lr")
                    zc = gp.tile([P, K, P], f16, tag="zc")
                    for j in range(nb4):
                        zb = zp.tile([P, 4, P], f32, tag="zb")
                        nc.tensor.matmul(zb, i128, glo[:, 4 * j:4 * j + 4, :],
                                         start=True, stop=False)
                        nc.tensor.matmul(zb, i128, ghi[:, 4 * j:4 * j + 4, :],
                                         start=False, stop=False)
                        nc.tensor.matmul(zb, xrT_all[:, b, :], irep_sb,
                                         start=False, stop=True)
                        lrj = lr[:, 4 * j:4 * j + 4, :]
                        if lrelu_act:
                            nc.scalar.activation(lrj, zb, AT.Prelu,
                                                 alpha=NEG_SLOPE)
                        else:
                            nc.vector.scalar_tensor_tensor(
                                out=lrj, in0=zb, scalar=NEG_SLOPE, in1=zb,
                                op0=OP.mult, op1=OP.max)
                        nc.scalar.copy(zc[:, 4 * j:4 * j + 4, :], zb)

                    if stage < 3:
                        nc.vector.tensor_copy(out_sb[:, b:b + 1],
                                              zc[:, 0, 0:1])
                        continue
                    lrv = lr.rearrange("p (a b) f -> p a (b f)", b=4)
                    att_b = bc(att_sb, [[0, nb4], [1, 4 * P]])
                    nc.vector.tensor_mul(lrv, lrv, att_b)
                    e_t = sp.tile([P, K], f32, tag="e")
                    nc.vector.reduce_sum(out=e_t, in_=lr, axis=AX.X)
                    pp_t = sp.tile([P, K], f16, tag="pp")
                    nc.scalar.activation(pp_t, e_t, AT.Exp)
                    pm = sp.tile([P, K], f16, tag="pm")
                    nc.vector.tensor_mul(pm, pp_t, mask_sb[:, ko:ko + K])
                    den = sp.tile([P, 1], f32, tag="den")
                    nc.vector.reduce_sum(out=den, in_=pm, axis=AX.X)
                    rden = sp.tile([P, 1], f32, tag="rden")
                    nc.vector.reciprocal(rden, den)
                    if stage < 4:
                        nc.vector.tensor_copy(out_sb[:, b:b + 1], rden)
                        continue
                    pmb = bc(pm, [[1, K], [0, P]])
                    nc.vector.tensor_mul(zc, zc, pmb)
                    agg = sp.tile([P, P], f32, tag="agg")
                    zcT = bc(zc, [[1, P], [P, K]])
                    nc.vector.reduce_sum(out=agg, in_=zcT, axis=AX.X)
                    h_t = sp.tile([P, P], f32, tag="h")
                    nc.vector.scalar_tensor_tensor(
                        out=h_t, in0=agg, scalar=rden, in1=xr_all[:, b, :],
                        op0=OP.mult, op1=OP.subtract)
                    scr = sp.tile([P, P], f32, tag="scr")
                    lg = sp.tile([P, 1], f32, tag="lg")
                    nc.vector.tensor_mul(scr, h_t, wo_sb)
                    nc.vector.reduce_sum(out=lg, in_=scr, axis=AX.X)
                    nc.scalar.activation(out_sb[:, b:b + 1], lg, AT.Sigmoid,
                                         bias=bo_sb)

            nc.sync.dma_start(
                out=out_d.ap().rearrange("(b n) o -> n (b o)", n=P),
                in_=out_sb)
    nc.compile()
    return nc


# --------------------------------------------------------------------------
# Entry point
# --------------------------------------------------------------------------

def _run(inputs, trace=False, lrelu_act=True):
    from concourse.bass_utils import run_bass_kernel_spmd

    cfg, in_maps, core_nodes = _plan(**inputs)
    nc = _build(cfg, lrelu_act=lrelu_act)
    res = run_bass_kernel_spmd(nc, in_maps, core_ids=list(range(cfg["C"])),
                               trace=trace)

    N = cfg["N"]
    out = np.zeros((N, 1), dtype=np.float32)
    for c in range(cfg["C"]):
        nodes = core_nodes[c]
        ok = nodes >= 0
        out[nodes[ok], 0] = res.results[c]["out"][ok, 0]
    return out, res


def kernel(**inputs):
    return _run(inputs)[0]
